# revision 2
# baseline (speedup 1.0000x reference)
"""Causal self-attention (B=4, T=2048, C=1024, H=16, D=64) on 8 TRN2 NeuronCores.

Sharding: core = (batch b, head-group g), b = core // 2, g = core % 2.
Each core computes heads [8g, 8g+8) of batch b and the partial out-projection
(C, T) for its head group; host sums the two partials per batch and adds bias.

v2 design (vs baseline):
- tt-outer pipelined loop: vproj slices, qk-proj+RoPE, scores+exp, att@V,
  normalize, transpose, out-proj and output DMA all advance per 512-t tile.
- scores run as fp8e4 DoubleRow matmuls with a zero-padded pair lane
  (contraction 64 real + 64 zeros) -> half cost per column.
- att@V runs in [t, d] output orientation (128 out partitions) with E as the
  stationary operand and V pairs along 256-deep s superblocks (DoubleRow).
  Denominator comes from a ones column in V; normalization is then a
  per-partition scalar multiply fused into the PSUM->SBUF copy.
- y is PE-transposed back to [hd, t] for the fp16 out-projection.
- v-bias is folded into out_b on the host (softmax weights sum to 1).
"""

import numpy as np

B, T, C = 4, 2048, 1024
H, D = 16, 64
N_CORES = 8
HPG = H // 2            # heads per core
NCHUNK = 4              # head-pair chunks per core
KT = 8                  # k-tiles of 128 over C
TT = 4                  # t-tiles of 512 over T
NT = 512
NM = 16                 # 128-row t slices over T
VS = 65                 # v cols per head (64 + ones)
VW = 528                # v block stride per 128-t slice (8*65 + 8 pad, %16==0)
ROPE_BASE = 10000.0

# dtype toggles
SCORES_DR = True        # q/k fp8e4 + DoubleRow zero-pad scores
ATTV_DR = False         # E,V fp8e4 + DoubleRow att@V (fp8 V costs ~2% err)
E_SCALE = 8.0           # E stored as exp(S/8)/E_SCALE in fp8e3m4
QK_FP8 = True           # x,wqk fp8 + DoubleRow qk projection
FASTEXP = True          # offload some exp tiles to DVE (Schraudolph-to-fp16)
FE_A = 184.6649652337873
FE_B = 15300.0


def _fe_on(c, tt, sc):
    if not FASTEXP:
        return False
    if tt == 3:
        return sc % 5 == 2
    if tt == 2:
        return sc % 9 == 4
    return False

_CACHE = {}


def _build_nc():
    import concourse.bass as bass  # noqa: F401
    import concourse.tile as tile
    from concourse import bacc, mybir
    from contextlib import ExitStack

    f16 = mybir.dt.float16
    f32 = mybir.dt.float32
    f8 = mybir.dt.float8e4
    DR = mybir.MatmulPerfMode.DoubleRow
    qk_dt = f8 if SCORES_DR else f16
    e_dt = mybir.dt.float8e4 if ATTV_DR else f16
    v_dt = f8 if ATTV_DR else f16

    nc = bacc.Bacc(
        "TRN2",
        target_bir_lowering=False,
        debug=False,
        enable_asserts=True,
        num_devices=N_CORES,
    )

    xt_d = nc.dram_tensor("xt", (KT * 128, T), f16, kind="ExternalInput").ap()
    if QK_FP8:
        xt8_d = nc.dram_tensor("xt8", (KT * 128, T), f8, kind="ExternalInput").ap()
    wqk_d = nc.dram_tensor("wqk", (128, KT * 1024), f8 if QK_FP8 else f16,
                           kind="ExternalInput").ap()
    wv_d = nc.dram_tensor("wv", (128, KT * 512), v_dt, kind="ExternalInput").ap()
    wo_d = nc.dram_tensor("wo", (128, NCHUNK * 1024), f16, kind="ExternalInput").ap()
    bqk_d = nc.dram_tensor("bqk", (128, 16), f32, kind="ExternalInput").ap()
    cs_d = nc.dram_tensor("cs", (128, T), f16, kind="ExternalInput").ap()
    css_d = nc.dram_tensor("css", (128, T), f16, kind="ExternalInput").ap()
    idn_d = nc.dram_tensor("idn", (128, 128), f16, kind="ExternalInput").ap()
    ot_d = nc.dram_tensor("ot", (1024, T), f16, kind="ExternalOutput").ap()

    SHUF = list(range(16, 32)) + list(range(0, 16))

    with tile.TileContext(nc) as tc:
        with ExitStack() as ctx, nc.allow_low_precision("fp16/fp8 attention"):
            consts = ctx.enter_context(tc.tile_pool(name="consts", bufs=1))
            rtmp = ctx.enter_context(tc.tile_pool(name="rtmp", bufs=2))
            e_pool = ctx.enter_context(tc.tile_pool(name="e", bufs=1))
            ysb_pool = ctx.enter_context(tc.tile_pool(name="ysb", bufs=2))
            yt_pool = ctx.enter_context(tc.tile_pool(name="yt", bufs=2))
            osb = ctx.enter_context(tc.tile_pool(name="osb", bufs=4))
            small = ctx.enter_context(tc.tile_pool(name="small", bufs=4))
            ps_big = ctx.enter_context(tc.tile_pool(name="psbig", bufs=2, space="PSUM"))
            ps_s = ctx.enter_context(tc.tile_pool(name="pss", bufs=2, space="PSUM"))
            ps_y = ctx.enter_context(tc.tile_pool(name="psy", bufs=1, space="PSUM"))

            # ---- resident tiles + input DMA ----
            # Ordered for earliest compute start: wv + first t-quarter of xt
            # (vproj m0..3), then wqk/tables (qk tt0), then the rest of xt.
            wv = consts.tile([128, KT * 512], v_dt)
            nc.sync.dma_start(wv[:], wv_d[:])
            xq_pool = ctx.enter_context(tc.tile_pool(name="xq", bufs=2))

            def load_xq(tt):
                xq = xq_pool.tile([128, KT, 512], f16, tag="xq", name="xq")
                for kc in range(KT):
                    nc.sync.dma_start(
                        xq[:, kc, :],
                        xt_d[kc * 128:(kc + 1) * 128, tt * 512:(tt + 1) * 512])
                return xq

            xq0 = load_xq(0)
            if QK_FP8:
                xt8 = consts.tile([128, KT * T], f8)
                for kc in range(KT):
                    nc.sync.dma_start(xt8[:, kc * T:(kc + 1) * T],
                                      xt8_d[kc * 128:(kc + 1) * 128, :])
                qk_src = xt8
            wqk = consts.tile([128, KT * 1024], f8 if QK_FP8 else f16)
            nw = 2 if QK_FP8 else 4
            for i in range(nw):
                nc.sync.dma_start(wqk[:, i * (KT * 1024 // nw):(i + 1) * (KT * 1024 // nw)],
                                  wqk_d[:, i * (KT * 1024 // nw):(i + 1) * (KT * 1024 // nw)])
            bqk = consts.tile([128, 16], f32)
            nc.sync.dma_start(bqk[:], bqk_d[:])
            cs = consts.tile([128, T], f16)
            css = consts.tile([128, T], f16)
            idn = consts.tile([128, 128], f16)
            if not QK_FP8:
                xt = consts.tile([128, KT * T], f16)
                for kc in range(KT):
                    nc.sync.dma_start(xt[:, kc * T:(kc + 1) * T],
                                      xt_d[kc * 128:(kc + 1) * 128, :])
                qk_src = xt
            nc.sync.dma_start(cs[:], cs_d[:])
            nc.sync.dma_start(css[:], css_d[:])
            nc.sync.dma_start(idn[:], idn_d[:])
            wo = consts.tile([128, NCHUNK * 1024], f16)
            for i in range(2):
                nc.sync.dma_start(wo[:, i * 2048:(i + 1) * 2048],
                                  wo_d[:, i * 2048:(i + 1) * 2048])

            # persistent activations
            v_sb = consts.tile([128, NM, 8, 66], v_dt)    # [s(128), m, h, j]
            rq = [[consts.tile([128, 2, NT], qk_dt, name=f"rq{c}_{par}")
                   for par in range(2)] for c in range(NCHUNK)]
            rk = [consts.tile([128, 2, T], qk_dt, name=f"rk{c}") for c in range(NCHUNK)]
            # E buffers: [s, h, j, t] per (chunk, superblock); chunks of the
            # same parity share SBUF buffers (tag rotation) so fp16 fits.
            e_cur = {}

            def e_tile(c, sb, create=False):
                if create:
                    e_cur[(c, sb)] = e_pool.tile(
                        [128, 2, 2, NT], e_dt, name=f"eb{c & 1}_{sb}",
                        tag=f"eb{c & 1}_{sb}")
                return e_cur[(c, sb)]

            # ones cols in V + zero pair lanes of rq/rk
            nc.gpsimd.memset(v_sb[:, :, :, 64:65], 1.0)
            if SCORES_DR:
                for c in range(NCHUNK):
                    nc.gpsimd.memset(rq[c][0][:, 1, :], 0)
                    nc.gpsimd.memset(rq[c][1][:, 1, :], 0)
                    nc.gpsimd.memset(rk[c][:, 1, :], 0)

            def vproj(m, xq):
                """Project V for t-slice m (128 t rows on partitions)."""
                psa = ps_big.tile([128, 512], f32, tag="big")
                mo = (m % 4) * 128
                for kc in range(KT):
                    nc.tensor.matmul(psa[:], xq[:, kc, mo:mo + 128],
                                     wv[:, kc * 512:(kc + 1) * 512],
                                     start=(kc == 0), stop=(kc == KT - 1))
                # scatter tight 512 -> strided head layout
                nc.vector.tensor_copy(
                    v_sb[:, m, :, 0:64],
                    psa[:].rearrange("p (h j) -> p h j", j=64))

            def qkrope(c, tt):
                t0 = tt * NT
                for which in range(2):
                    ps = ps_big.tile([128, 512], f32, tag="big")
                    if QK_FP8:
                        for k2 in range(KT // 2):
                            lhsT = wqk[:].rearrange(
                                "p (kc m) -> p kc m", m=1024)[:, 2 * k2:2 * k2 + 2,
                                                              c * 256 + which * 128:
                                                              c * 256 + which * 128 + 128]
                            rhs = qk_src[:].rearrange(
                                "p (kc t) -> p kc t", t=T)[:, 2 * k2:2 * k2 + 2,
                                                           t0:t0 + NT]
                            nc.tensor.matmul(ps[:], lhsT, rhs,
                                             start=(k2 == 0), stop=(k2 == KT // 2 - 1),
                                             perf_mode=DR)
                    else:
                        for kc in range(KT):
                            lhsT = wqk[:, kc * 1024 + c * 256 + which * 128:
                                       kc * 1024 + c * 256 + which * 128 + 128]
                            nc.tensor.matmul(ps[:], lhsT,
                                             qk_src[:, kc * T + t0:kc * T + t0 + NT],
                                             start=(kc == 0), stop=(kc == KT - 1))
                    bcol = bqk[:, c * 4 + which * 2:c * 4 + which * 2 + 1]
                    bswp = bqk[:, c * 4 + which * 2 + 1:c * 4 + which * 2 + 2]
                    s16 = rtmp.tile([128, 512], f32, tag="s16")
                    nc.vector.stream_shuffle(s16[:], ps[:], SHUF)
                    x1 = rtmp.tile([128, 512], f16, tag="x1")
                    nc.vector.scalar_tensor_tensor(
                        out=x1[:], in0=ps[:], scalar=bcol, in1=cs[:, t0:t0 + NT],
                        op0=mybir.AluOpType.add, op1=mybir.AluOpType.mult)
                    x2 = rtmp.tile([128, 512], f16, tag="x2")
                    nc.vector.scalar_tensor_tensor(
                        out=x2[:], in0=s16[:], scalar=bswp, in1=css[:, t0:t0 + NT],
                        op0=mybir.AluOpType.add, op1=mybir.AluOpType.mult)
                    dst = (rq[c][tt & 1][:, 0, :] if which == 0
                           else rk[c][:, 0, t0:t0 + NT])
                    nc.vector.tensor_add(dst, x1[:], x2[:])

            def score_step(c, tt, sc):
                t0 = tt * NT
                if True:
                    s0 = sc * 128
                    dlt = max(0, s0 - t0)
                    sp = ps_s.tile([128, 2, 512], f32, tag="s")
                    for h in range(2):
                        if SCORES_DR:
                            nc.tensor.matmul(
                                sp[:, h, dlt:NT],
                                rk[c][64 * h:64 * h + 64, :, s0:s0 + 128],
                                rq[c][tt & 1][64 * h:64 * h + 64, :, dlt:NT],
                                start=True, stop=True, perf_mode=DR,
                                skip_group_check=True)
                        else:
                            nc.tensor.matmul(
                                sp[:, h, dlt:NT],
                                rk[c][64 * h:64 * h + 64, 0, s0:s0 + 128],
                                rq[c][tt & 1][64 * h:64 * h + 64, 0, dlt:NT],
                                start=True, stop=True, skip_group_check=True)
                    sb, j = sc // 2, sc % 2
                    eb = e_tile(c, sb, create=(j == 0))
                    e3 = eb[:, :, j, dlt:NT]
                    if _fe_on(c, tt, sc):
                        nc.vector.tensor_scalar(
                            out=e3.bitcast(mybir.dt.int16), in0=sp[:, :, dlt:NT],
                            scalar1=FE_A, scalar2=FE_B,
                            op0=mybir.AluOpType.mult, op1=mybir.AluOpType.add)
                    else:
                        nc.scalar.activation(e3, sp[:, :, dlt:NT],
                                             mybir.ActivationFunctionType.Exp,
                                             bias=0.0, scale=0.125)
                    if s0 >= t0:  # diagonal block: zero upper triangle
                        nc.gpsimd.affine_select(
                            out=eb[:, :, j, dlt:dlt + 128],
                            in_=eb[:, :, j, dlt:dlt + 128],
                            compare_op=mybir.AluOpType.is_ge,
                            fill=0.0, base=0,
                            pattern=[[0, 2], [1, 128]], channel_multiplier=-1)

            y_sb_map = {}
            yT_map = {}

            def attv_half(tt, tblk, half):
                """y[t, 65] accumulation for heads of chunks (2*half, 2*half+1)."""
                tba = 4 * tt + tblk
                n = tba + 1                      # valid 128-s blocks
                tc0 = tblk * 128
                y_t = ps_y.tile([128, 4, VS], f32, tag=f"y{half}")
                for hh4 in range(4):             # head within half
                    hh = 4 * half + hh4
                    c, h = hh // 2, hh % 2
                    first = True
                    if ATTV_DR:
                        for sb in range(n // 2):
                            nc.tensor.matmul(
                                y_t[:, hh4, :],
                                e_tile(c, sb)[:, h, :, tc0:tc0 + 128],
                                v_sb[:, 2 * sb:2 * sb + 2, hh, 0:VS],
                                start=first, stop=(n % 2 == 0 and sb == n // 2 - 1),
                                perf_mode=DR, skip_group_check=True)
                            first = False
                        if n % 2 == 1:
                            sb = n // 2
                            nc.tensor.matmul(
                                y_t[:, hh4, :],
                                e_tile(c, sb)[:, h, 0, tc0:tc0 + 128],
                                v_sb[:, 2 * sb, hh, 0:VS],
                                start=first, stop=True, skip_group_check=True)
                    else:
                        for m in range(n):
                            nc.tensor.matmul(
                                y_t[:, hh4, :],
                                e_tile(c, m // 2)[:, h, m % 2, tc0:tc0 + 128],
                                v_sb[:, m, hh, 0:VS],
                                start=(m == 0), stop=(m == n - 1),
                                skip_group_check=True)
                return y_t

            def norm_half(y_t, y_sb, half):
                rd = small.tile([128, 4], f32, tag="rd")
                nc.vector.reciprocal(rd[:], y_t[:, :, 64])
                nc.vector.scalar_tensor_tensor(
                    out=y_sb[:, 4 * half:4 * half + 4, :],
                    in0=y_t[:, :, 0:64], scalar=1.0,
                    in1=rd[:].unsqueeze(2).broadcast_to((128, 4, 64)),
                    op0=mybir.AluOpType.mult, op1=mybir.AluOpType.mult)

            def attv_granule(tt, tblk, half):
                if half == 0:
                    y_sb = ysb_pool.tile([128, 8, 64], f16, tag=f"ysb{tblk}",
                                         name=f"ysb{tblk}")
                    y_sb_map[(tt, tblk)] = y_sb
                y_sb = y_sb_map[(tt, tblk)]
                y_t = attv_half(tt, tblk, half)
                norm_half(y_t, y_sb, half)

            def transp_granule(tt, tblk, c):
                if (tt, c) not in yT_map:
                    yT_map[(tt, c)] = yt_pool.tile([128, 512], f16, tag=f"yT{c}",
                                                   name=f"yT{c}")
                y_sb = y_sb_map[(tt, tblk)]
                ptr = ps_big.tile([128, 128], f16, tag="big")
                nc.tensor.transpose(
                    ptr[:], y_sb[:, 2 * c:2 * c + 2, :].rearrange("p h j -> p (h j)"),
                    idn[:])
                nc.vector.tensor_copy(yT_map[(tt, c)][:, tblk * 128:(tblk + 1) * 128],
                                      ptr[:])

            def oproj_granule(tt, ct):
                t0 = tt * NT
                po = ps_big.tile([128, 512], f32, tag="big")
                for c in range(NCHUNK):
                    nc.tensor.matmul(po[:], wo[:, c * 1024 + ct * 128:c * 1024 + ct * 128 + 128],
                                     yT_map[(tt, c)][:], start=(c == 0), stop=(c == NCHUNK - 1))
                ob = osb.tile([128, 512], f16)
                nc.vector.tensor_copy(ob[:], po[:])
                nc.sync.dma_start(ot_d[ct * 128:(ct + 1) * 128, t0:t0 + NT], ob[:])

            # ---- main pipelined loop ----
            # Per tt: walk the score steps of the 4 chunks; interleave
            # (a) vproj slices for this tt (early), (b) deferred transpose +
            # out-proj granules of tt-1 (proportional), (c) att@V granules of
            # this tt as soon as the needed chunks' exps are emitted.
            deferred = []
            for tt in range(TT):
                nsc = 4 * (tt + 1)
                xq = xq0 if tt == 0 else xq_next  # noqa
                fill_early = [(vproj, (m, xq)) for m in range(4 * tt, 4 * tt + 4)]
                fill_late = list(deferred)
                nsteps = 4 * nsc
                n_fill = len(fill_late)
                step = 0
                popped = 0
                with nc.named_scope(f"tt{tt}"):
                    qkrope(0, tt)
                    qkrope(1, tt)
                    for c in range(NCHUNK):
                        if c >= 1 and c + 1 < NCHUNK:
                            qkrope(c + 1, tt)
                        for sc in range(nsc):
                            score_step(c, tt, sc)
                            step += 1
                            # early fill: drain vproj within first chunk walk
                            vp_target = min(4, step * 4 // nsc + 1)
                            while fill_early and 4 - len(fill_early) < vp_target:
                                f, a = fill_early.pop(0)
                                f(*a)
                            # proportional late fill
                            tgt = n_fill * step // nsteps
                            while popped < tgt and fill_late:
                                f, a = fill_late.pop(0)
                                f(*a)
                                popped += 1
                            # att@V: half A as c1's diagonal exps land; B at end
                            if c == 1 and sc >= nsc - 4:
                                attv_granule(tt, sc - (nsc - 4), 0)
                            if c == 3 and sc >= nsc - 4:
                                attv_granule(tt, sc - (nsc - 4), 1)
                    while fill_early:
                        f, a = fill_early.pop(0)
                        f(*a)
                    while fill_late:
                        f, a = fill_late.pop(0)
                        f(*a)
                if tt + 1 < TT:
                    xq_next = load_xq(tt + 1)
                deferred = [(transp_granule, (tt, tblk, c))
                            for tblk in range(4) for c in range(NCHUNK)]
                deferred += [(oproj_granule, (tt, ct)) for ct in range(8)]
            with nc.named_scope("tail"):
                for f, a in deferred:
                    f(*a)

    nc.compile()
    return nc


def _prep_inputs(x, qkv_w, qkv_b):
    import ml_dtypes
    f8_np = ml_dtypes.float8_e4m3
    ev_np = f8_np if ATTV_DR else np.float16
    qk_np = f8_np if QK_FP8 else np.float16
    qk_scale = 16.0 if QK_FP8 else 1.0
    x = np.asarray(x, dtype=np.float32)
    qkv_w = np.asarray(qkv_w, dtype=np.float32)
    qkv_b = np.asarray(qkv_b, dtype=np.float32)

    xts = []
    xt8s = []
    for b in range(B):
        xts.append(np.ascontiguousarray(x[b].T).astype(np.float16))  # (1024, T)
        xt8s.append(np.ascontiguousarray(x[b].T).astype(np.float16).astype(f8_np))

    r = np.arange(64)
    d_r = 2 * ((r // 32) * 16 + (r % 16)) + ((r % 32) >= 16)
    p = np.arange(128)
    f_p = ((p // 32) % 2) * 16 + (p % 16)

    ins_g = []
    for g in range(2):
        wqk = np.empty((128, KT * 1024), dtype=qk_np)
        bqk = np.empty((128, 16), dtype=np.float32)
        for c in range(NCHUNK):
            for which in range(2):
                rows = np.concatenate([
                    which * C + (8 * g + 2 * c + hh) * 64 + d_r for hh in range(2)
                ])
                blk = qkv_w[rows, :]
                for kc in range(KT):
                    wqk[:, kc * 1024 + c * 256 + which * 128:
                        kc * 1024 + c * 256 + which * 128 + 128] = \
                        (blk[:, kc * 128:(kc + 1) * 128].T * qk_scale).astype(qk_np)
                bc = qkv_b[rows].astype(np.float32) * qk_scale
                bqk[:, c * 4 + which * 2] = bc
                bqk[:, c * 4 + which * 2 + 1] = bc[p ^ 16]
        # wv tight: [128 (kc rows), kc*512 + 64h + j], no bias (folded to out_b)
        wva = np.empty((KT * 128, 512), dtype=np.float32)
        for h in range(HPG):
            rows = 2 * C + (8 * g + h) * 64 + np.arange(64)
            wva[:, 64 * h:64 * h + 64] = qkv_w[rows, :].T
        wv = np.empty((128, KT * 512), dtype=ev_np)
        for kc in range(KT):
            wv[:, kc * 512:(kc + 1) * 512] = wva[kc * 128:(kc + 1) * 128].astype(ev_np)
        ins_g.append((wqk, bqk, wv))

    inv_freq = (1.0 / (ROPE_BASE ** (np.arange(0, D, 2) / D))).astype(np.float64)
    t = np.arange(T, dtype=np.float64)
    ang = t[None, :] * inv_freq[f_p][:, None]
    cs = (np.cos(ang) / qk_scale).astype(np.float16)
    sgn = np.where((p % 32) < 16, -1.0, 1.0)[:, None]
    css = (sgn * np.sin(ang) / qk_scale).astype(np.float16)
    idn = np.eye(128, dtype=np.float16)

    return xts, xt8s, ins_g, cs, css, idn


def _prep_wo(out_w, g):
    out_w = np.asarray(out_w, dtype=np.float32)
    wo = np.empty((128, NCHUNK * 1024), dtype=np.float16)
    for c in range(NCHUNK):
        rows = np.concatenate([(8 * g + 2 * c + hh) * 64 + np.arange(64) for hh in range(2)])
        wo[:, c * 1024:(c + 1) * 1024] = out_w[:, rows].astype(np.float16).T
    return wo


def kernel(x, qkv_w, qkv_b, out_w, out_b):
    from concourse.bass_utils import run_bass_kernel_spmd

    if "nc" not in _CACHE:
        _CACHE["nc"] = _build_nc()
    nc = _CACHE["nc"]

    xts, xt8s, ins_g, cs, css, idn = _prep_inputs(x, qkv_w, qkv_b)
    wos = [_prep_wo(out_w, g) for g in range(2)]
    out_w = np.asarray(out_w, dtype=np.float32)
    qkv_b = np.asarray(qkv_b, dtype=np.float32)
    out_b = np.asarray(out_b, dtype=np.float32) + out_w @ qkv_b[2 * C:]

    in_maps = []
    for core in range(N_CORES):
        b, g = core // 2, core % 2
        wqk, bqk, wv = ins_g[g]
        im = {
            "xt": xts[b], "wqk": wqk, "wv": wv, "wo": wos[g],
            "bqk": bqk, "cs": cs, "css": css, "idn": idn,
        }
        if QK_FP8:
            im["xt8"] = xt8s[b]
        in_maps.append(im)

    try:
        res = run_bass_kernel_spmd(nc, in_maps, core_ids=list(range(N_CORES)))
    except ModuleNotFoundError:
        import os
        os.environ["BASS_NEVER_TRACE"] = "1"
        res = run_bass_kernel_spmd(nc, in_maps, core_ids=list(range(N_CORES)))

    out = np.empty((B, T, C), dtype=np.float32)
    for b in range(B):
        pt = res.results[2 * b]["ot"].astype(np.float32) + \
            res.results[2 * b + 1]["ot"].astype(np.float32)
        out[b] = pt.T + out_b[None, :]
    return out
